# revision 1
# baseline (speedup 1.0000x reference)
"""Trainium2 Bass kernel for nn_HeatmapBatch.

Reference computes: one-hot delta (value 10.0) per (batch, keypoint) at
integer coords (r, c) in a 256x256 image, then depthwise-convolves with a
shared 9x9 kernel.  Since each image holds exactly one delta, the output is
zeros everywhere except a 9x9 patch of 10*kernel2d[::-1,::-1] (XLA conv is
cross-correlation) centred at (r, c), clipped at the borders.

Device strategy (data-parallel over batch, 8 cores x 8 batches = 168
images per core):
  - Output per core is a column-padded [168*256 (+dump), 264] f32 tensor
    (4 pad columns each side) so a patch row never wraps to the next row.
  - The runtime hands kernels pre-zeroed ExternalOutput buffers (documented
    contract in bass_utils/bass2jax: "kernels that don't write every element
    rely on that"), so the kernel only scatters the patches.
  - A whole unclipped patch (rows r-4..r+4) is one contiguous 2121-element
    span of the padded image: 9 K-rows separated by 255 zeros.  Overwriting
    those gap zeros with zeros is harmless, so one indirect-DMA descriptor
    per patch suffices: 2 scatter calls cover 168 patches (126+42
    partitions).  Patches clipped at the top/bottom border are redirected
    to a dump zone and their valid rows written by a third scatter call
    (9-element chunks, dump-padded slots).
  - Scatter indices are host-fused from x (the sharding hint's "fused
    batch*kp scatter indices"); all value math (10*kernel) runs on device.
  - A fallback variant zero-fills the output with big DMAs first, in case
    the pre-zeroed-output contract ever fails (detected by sampling), and a
    12-call row-scatter variant covers the (practically impossible) case of
    more than 126 clipped rows per core.
Host does sharding/layout prep and the final gather/strip of the padding.
"""

import numpy as np


def _ensure_axon_hooks():
    """bass_utils imports antenv.axon_hooks when tracing is requested (e.g.
    BASS_TRACE=1 in the environment); some images lack that module.  Provide
    it best-effort so a tracing harness degrades gracefully instead of
    crashing.  Never raises."""
    try:
        import antenv.axon_hooks  # noqa: F401
        return
    except Exception:
        pass
    try:
        import sys
        import types

        import antenv

        mod = types.ModuleType("antenv.axon_hooks")
        _state = {"hook": None}
        mod.set_axon_ntff_profile_hook = lambda h: _state.__setitem__("hook", h)
        mod.get_axon_ntff_profile_hook = lambda: _state["hook"]
        sys.modules["antenv.axon_hooks"] = mod
        antenv.axon_hooks = mod
        try:
            from trn_agent_boot.trn_boot import _ntff_profile_via_ctypes

            mod.set_axon_ntff_profile_hook(
                _ntff_profile_via_ctypes("/opt/axon/libaxon_pjrt.so")
            )
        except Exception:
            pass
    except Exception:
        pass


_ensure_axon_hooks()

B, KP, H = 64, 21, 256
KS, PAD = 9, 4
NCORES = 8
BLOC = B // NCORES          # 8 batches per core
NPTS = BLOC * KP            # 168 images per core
QP = 126                    # partitions used per scatter call
WPAD = H + 2 * PAD          # 264 padded columns
ROWS = NPTS * H             # 43008 image rows per core
PATCH = 8 * WPAD + KS       # 2121: contiguous span of one unclipped patch
# Dump zone: redirected writes must not collide (same-address sub-512B HBM
# writes serialize as read-modify-writes), so every dump write gets its own
# region: 16 patch-sized slots + 126 row-sized slots.
NPDUMP = 16
DROWS = (NPDUMP * PATCH + QP * KS + WPAD - 1) // WPAD + 1   # 134 rows
OROWS = ROWS + DROWS        # output rows incl. dump zone
DUMP = ROWS * WPAD          # first element of the dump zone
RDUMP = DUMP + NPDUMP * PATCH   # row-slot dump area

_NC_CACHE = {}


def _build_nc(mode: str, zero_fill: bool):
    from concourse import bass, bacc, mybir
    import concourse.tile as tile

    nc = bacc.Bacc(None, target_bir_lowering=False)
    i32, f32 = mybir.dt.int32, mybir.dt.float32
    out = nc.dram_tensor("out", [OROWS, WPAD], f32, kind="ExternalOutput")

    if mode == "patch3":
        idxs = nc.dram_tensor("idxs", [QP, 3], i32, kind="ExternalInput")
        kvals = nc.dram_tensor("kvals", [QP, 90], f32, kind="ExternalInput")
    else:  # rows12: one 9-elem segment per patch-row slot
        idxs = nc.dram_tensor("idxs", [QP, 12], i32, kind="ExternalInput")
        kvals = nc.dram_tensor("kvals", [QP, 108], f32, kind="ExternalInput")

    with tile.TileContext(nc) as tc:
        with tc.tile_pool(name="sbuf", bufs=1) as pool:
            if mode == "patch3":
                idx_t = pool.tile([QP, 3], i32)
                kv_t = pool.tile([QP, 90], f32)
            else:
                idx_t = pool.tile([QP, 12], i32)
                kv_t = pool.tile([QP, 108], f32)
            if mode == "patch3":
                pbuf = pool.tile([QP, PATCH], f32)
                nc.vector.memset(pbuf[:], 0.0)
            nc.sync.dma_start(out=idx_t[:], in_=idxs[:])
            nc.sync.dma_start(out=kv_t[:], in_=kvals[:])

            if zero_fill:
                zt = pool.tile([128, 2772], mybir.dt.float32)
                nc.vector.memset(zt[:], 0.0)
                blk = 1344  # 1344*264*4B = 1.42 MB per fill DMA
                for i in range(ROWS // blk):
                    nc.sync.dma_start(
                        out=out[i * blk:(i + 1) * blk, :], in_=zt[:, :]
                    )
                nc.sync.dma_start(
                    out=out[ROWS:ROWS + 128, :], in_=zt[:, :WPAD]
                )
                nc.sync.dma_start(
                    out=out[ROWS + 128:OROWS, :],
                    in_=zt[:DROWS - 128, :WPAD],
                )

            if mode == "patch3":
                rbuf = pool.tile([QP, KS], f32)
                for k in range(KS):
                    nc.vector.tensor_scalar_mul(
                        pbuf[:, k * WPAD:k * WPAD + KS],
                        kv_t[:, k * KS:(k + 1) * KS],
                        10.0,
                    )
                nc.vector.tensor_scalar_mul(rbuf[:], kv_t[:, 81:90], 10.0)
                for ap_in, ap_idx in (
                    (pbuf[:], idx_t[:, 0:1]),
                    (pbuf[:42, :], idx_t[:42, 1:2]),
                    (rbuf[:], idx_t[:, 2:3]),
                ):
                    nc.gpsimd.indirect_dma_start(
                        out=out[:],
                        out_offset=bass.IndirectOffsetOnAxis(ap=ap_idx, axis=1),
                        in_=ap_in,
                        in_offset=None,
                    )
            else:
                k10 = pool.tile([QP, 12, KS], f32)
                nc.vector.tensor_scalar_mul(k10[:], kv_t[:], 10.0)
                for j in range(12):
                    nc.gpsimd.indirect_dma_start(
                        out=out[:],
                        out_offset=bass.IndirectOffsetOnAxis(
                            ap=idx_t[:, j:j + 1], axis=1
                        ),
                        in_=k10[:, j, :],
                        in_offset=None,
                    )
    return nc


def _build_nc_raw():
    """patch3 fast path in raw Bass: manual semaphores, no conservative
    inter-call serialization — the three indirect DMAs issue back-to-back
    and one final wait covers all completions."""
    from concourse import bass, mybir

    nc = bass.Bass(target_bir_lowering=False)
    i32, f32 = mybir.dt.int32, mybir.dt.float32
    out = nc.dram_tensor("out", [OROWS, WPAD], f32, kind="ExternalOutput")
    idxs = nc.dram_tensor("idxs", [QP, 3], i32, kind="ExternalInput")
    kvals = nc.dram_tensor("kvals", [QP, 90], f32, kind="ExternalInput")

    with (
        nc.Block() as block,
        nc.semaphore("s_in") as s_in,
        nc.semaphore("s_ix") as s_ix,
        nc.semaphore("s_v") as s_v,
        nc.semaphore("s_d") as s_d,
        nc.sbuf_tensor("idx_t", [QP, 3], i32) as idx_t,
        nc.sbuf_tensor("kv_t", [QP, 90], f32) as kv_t,
        nc.sbuf_tensor("pbuf", [QP, PATCH], f32) as pbuf,
        nc.sbuf_tensor("rbuf", [QP, KS], f32) as rbuf,
    ):

        @block.sync
        def _(sync):
            sync.dma_start(out=kv_t[:], in_=kvals[:]).then_inc(s_in, 16)
            sync.dma_start(out=idx_t[:], in_=idxs[:]).then_inc(s_ix, 16)

        @block.vector
        def _(vector):
            # zero only the inter-row gaps; the 9 K-row slots are written by
            # the scale-copies below, so all DVE writes stay disjoint
            vector.memset(
                bass.AP(pbuf, KS, [[PATCH, QP], [WPAD, KS - 1], [1, WPAD - KS]]),
                0.0,
            )
            vector.wait_ge(s_in, 16)
            vector.tensor_scalar_mul(rbuf[:], kv_t[:, 81:90], 10.0).then_inc(
                s_v, 1
            )
            for k in range(KS):
                ts = vector.tensor_scalar_mul(
                    pbuf[:, k * WPAD:k * WPAD + KS],
                    kv_t[:, k * KS:(k + 1) * KS],
                    10.0,
                )
            ts.then_inc(s_v, 1)

        @block.gpsimd
        def _(g):
            g.wait_ge(s_ix, 16)
            g.wait_ge(s_v, 1)
            # clip-row call first: its sub-512B RMW writes are the slowest
            # to land, so let them drain behind the patch calls' gen
            g.indirect_dma_start(
                out=out[:],
                out_offset=bass.IndirectOffsetOnAxis(ap=idx_t[:, 2:3], axis=1),
                in_=rbuf[:],
                in_offset=None,
            ).then_inc(s_d, 16)
            g.wait_ge(s_v, 2)
            g.indirect_dma_start(
                out=out[:],
                out_offset=bass.IndirectOffsetOnAxis(ap=idx_t[:, 0:1], axis=1),
                in_=pbuf[:],
                in_offset=None,
            ).then_inc(s_d, 16)
            g.indirect_dma_start(
                out=out[:],
                out_offset=bass.IndirectOffsetOnAxis(ap=idx_t[:42, 1:2], axis=1),
                in_=pbuf[:42, :],
                in_offset=None,
            ).then_inc(s_d, 16)
            g.wait_ge(s_d, 48)

    return nc


def _get_nc(mode: str, zero_fill: bool):
    key = (mode, zero_fill)
    if key not in _NC_CACHE:
        if mode == "patch3" and not zero_fill:
            nc = _build_nc_raw()
        else:
            nc = _build_nc(mode, zero_fill)
        if not nc.is_finalized():
            nc.finalize()
        _NC_CACHE[key] = nc
    return _NC_CACHE[key]


def _prep_patch3(xc, flip):
    """Host-fused indices + kernel-value tables for one core (mode patch3).

    Returns (idxs[126,3] i32, kvals[126,90] f32) or None if the clip call
    would overflow its 126 slots (fall back to rows12 then).
    """
    # default: every slot dumps to its own collision-free region
    idxs = np.empty((QP, 3), np.int32)
    idxs[:, 0] = DUMP + (np.arange(QP) % NPDUMP) * PATCH
    idxs[:, 1] = DUMP + (np.arange(QP) % NPDUMP) * PATCH
    idxs[:, 2] = RDUMP + np.arange(QP) * KS
    kvals = np.zeros((QP, 90), np.float32)
    kvals[:, :81] = flip.reshape(-1)[None, :]
    clip_i = []
    clip_k = []
    ndump = 0
    for p in range(NPTS):
        r, c = int(xc[p, 0]), int(xc[p, 1])
        start = WPAD * (H * p + r - PAD) + c
        if PAD <= r <= H - 1 - PAD:
            if p < QP:
                idxs[p, 0] = start
            else:
                idxs[p - QP, 1] = start
        else:
            ndump += 1
            for t in range(KS):
                rp = r - PAD + t
                if 0 <= rp < H:
                    clip_i.append(WPAD * (H * p + rp) + c)
                    clip_k.append(flip[t])
    if len(clip_i) > QP or ndump > NPDUMP:
        return None
    if clip_i:
        idxs[: len(clip_i), 2] = clip_i
        kvals[: len(clip_k), 81:90] = clip_k
    return idxs, kvals


_Q = np.arange(QP)
_TQ = _Q % KS
_P12 = 14 * np.arange(12)[None, :] + (_Q // KS)[:, None]   # [126,12] point id


def _prep_rows12(xc, flip):
    """Host-fused indices for the 12-call row-scatter fallback."""
    r = xc[_P12, 0].astype(np.int64)
    c = xc[_P12, 1].astype(np.int64)
    rp = r + _TQ[:, None] - PAD
    sidx = WPAD * (H * _P12 + rp) + c
    slot = (_Q[:, None] * 12 + np.arange(12)[None, :]) % (QP * 12)
    dump = DUMP + (slot % ((DROWS * WPAD) // KS - 1)) * KS
    sidx = np.where((rp < 0) | (rp >= H), dump, sidx).astype(np.int32)
    kvals = np.ascontiguousarray(
        np.broadcast_to(flip[_TQ][:, None, :], (QP, 12, KS))
    ).reshape(QP, 108).astype(np.float32)
    return sidx, kvals


def _in_maps(x, kernel2d):
    x = np.asarray(x)
    flip = np.asarray(kernel2d, dtype=np.float32)[::-1, ::-1]
    xr = x.reshape(NCORES, NPTS, 2)
    preps = [_prep_patch3(xr[c], flip) for c in range(NCORES)]
    if all(p is not None for p in preps):
        mode = "patch3"
        maps = [{"idxs": p[0], "kvals": p[1]} for p in preps]
    else:
        mode = "rows12"
        maps = []
        for c in range(NCORES):
            sidx, kvals = _prep_rows12(xr[c], flip)
            maps.append({"idxs": sidx, "kvals": kvals})
    return mode, maps


def _assemble(results):
    full = np.empty((B, KP, H, H), np.float32)
    for c, res in enumerate(results):
        o = res["out"][:ROWS].reshape(BLOC, KP, H, WPAD)
        full[c * BLOC:(c + 1) * BLOC] = o[:, :, :, PAD:PAD + H]
    return full


def _run(mode, zero_fill, maps, **kw):
    from concourse.bass_utils import run_bass_kernel_spmd

    nc = _get_nc(mode, zero_fill)
    return run_bass_kernel_spmd(nc, maps, core_ids=list(range(NCORES)), **kw)


def _zero_contract_ok(x, results):
    """Sample must-be-zero cells to confirm outputs arrived pre-zeroed."""
    x = np.asarray(x).reshape(NCORES, NPTS, 2)
    rng = np.random.RandomState(0)
    for c in (0, NCORES - 1):
        o = results[c]["out"][:ROWS].reshape(NPTS, H, WPAD)
        for p in rng.choice(NPTS, 24, replace=False):
            r = x[c, p, 0]
            rows = np.arange(H)
            far = rows[(rows < r - PAD - 1) | (rows > r + PAD + 1)]
            sel = rng.choice(far, 8, replace=False)
            if np.any(o[p][sel] != 0.0):
                return False
    return True


def kernel(x, kernel2d):
    mode, maps = _in_maps(x, kernel2d)
    res = _run(mode, False, maps)
    if not _zero_contract_ok(x, res.results):
        # pre-zeroed-output contract failed; redo with explicit zero fill
        res = _run(mode, True, maps)
    return _assemble(res.results)



# revision 5
# speedup vs baseline: 1.1527x; 1.1527x over previous
"""Trainium2 Bass kernel for nn_HeatmapBatch.

Reference computes: one-hot delta (value 10.0) per (batch, keypoint) at
integer coords (r, c) in a 256x256 image, then depthwise-convolves with a
shared 9x9 kernel.  Since each image holds exactly one delta, the output is
zeros everywhere except a 9x9 patch of 10*kernel2d[::-1,::-1] (XLA conv is
cross-correlation) centred at (r, c), clipped at the borders.

Device strategy (data-parallel over batch, 8 cores x 8 batches = 168
images per core):
  - Output per core is a column-padded [168*256 (+dump), 264] tensor in
    FP16 (rel tolerance is 2e-2; fp16 rounds at ~5e-4) so the scatter
    moves half the bytes; 4 pad columns each side so a patch row never
    wraps to the next row.  Host converts back to f32 and strips padding.
  - The runtime hands kernels pre-zeroed ExternalOutput buffers (documented
    contract in bass_utils/bass2jax), so the kernel only scatters patches.
  - A whole unclipped patch (rows r-4..r+4) is one contiguous 2121-element
    span of the padded image: 9 K-rows separated by 255 zeros.  Overwriting
    those gap zeros with zeros is harmless, so one indirect-DMA descriptor
    per patch suffices (the HW DGE pairs ONE offset per SBUF partition):
    2 scatter calls cover 168 patches (126+42 partitions).  Patches clipped
    at the top/bottom border are redirected to per-patch dump slots and
    their valid rows written by a third scatter call (9-element chunks,
    dump-padded slots; more calls only if >126 clipped rows, which needs
    >31 clipped points and never happens for uniform coords).
  - The ENTIRE span content (gap zeros + 10*flip value slots) is built on
    host and DMA'd straight into SBUF, so no engine ever touches the data
    between the input DMA and the scatter: no vector ops, no memset, no
    SBUF read-after-DMA races (the only consumers are the scatter packets,
    which start >1.3us after the descriptor-gen waits on the DMA sem).
  - Input DMAs issue in parallel (idx on the sync HW-DGE queue, span table
    on the scalar HW-DGE queue); desc-gen waits idx first, then the bigger
    table sem, so idx data has extra landing slack before the DGE reads it.
  - A fallback variant zero-fills the output with big DMAs first, in case
    the pre-zeroed-output contract ever fails (detected by sampling).
Host does sharding/layout prep and the final gather/strip of the padding.
"""

import numpy as np


def _ensure_axon_hooks():
    """bass_utils imports antenv.axon_hooks when tracing is requested (e.g.
    BASS_TRACE=1 in the environment); some images lack that module.  Provide
    it best-effort so a tracing harness degrades gracefully instead of
    crashing.  Never raises."""
    try:
        import antenv.axon_hooks  # noqa: F401
        return
    except Exception:
        pass
    try:
        import sys
        import types

        import antenv

        mod = types.ModuleType("antenv.axon_hooks")
        _state = {"hook": None}
        mod.set_axon_ntff_profile_hook = lambda h: _state.__setitem__("hook", h)
        mod.get_axon_ntff_profile_hook = lambda: _state["hook"]
        sys.modules["antenv.axon_hooks"] = mod
        antenv.axon_hooks = mod
        try:
            from trn_agent_boot.trn_boot import _ntff_profile_via_ctypes

            mod.set_axon_ntff_profile_hook(
                _ntff_profile_via_ctypes("/opt/axon/libaxon_pjrt.so")
            )
        except Exception:
            pass
    except Exception:
        pass


_ensure_axon_hooks()

B, KP, H = 64, 21, 256
KS, PAD = 9, 4
NCORES = 8
BLOC = B // NCORES          # 8 batches per core
NPTS = BLOC * KP            # 168 images per core
QP = 126                    # partitions used per scatter call
WPAD = H + 2 * PAD          # 264 padded columns
ROWS = NPTS * H             # 43008 image rows per core
SPAN = 8 * WPAD + KS        # 2121: contiguous span of one unclipped patch
NPDUMP = 32                 # patch-sized dump slots (distinct: parallel HBM)
DUMP = ROWS * WPAD          # first element of the dump zone
RDUMP = DUMP + NPDUMP * SPAN    # row-slot dump area

_NC_CACHE = {}


def _layout(n_clip_cols):
    dump_elems = NPDUMP * SPAN + n_clip_cols * QP * KS
    drows = (dump_elems + WPAD - 1) // WPAD
    return ROWS + drows


def _build_nc(zero_fill: bool, n_clip_cols: int):
    """Raw Bass: parallel input DMAs, span scatter (126+42) + n_clip_cols
    clip-row scatter calls, manual semaphores.  No compute engines."""
    from concourse import bass, mybir

    nc = bass.Bass(target_bir_lowering=False)
    i32, f16 = mybir.dt.int32, mybir.dt.float16
    orows = _layout(n_clip_cols)
    ncc = n_clip_cols
    width = SPAN + KS * ncc
    out = nc.dram_tensor("out", [orows, WPAD], f16, kind="ExternalOutput")
    idxs = nc.dram_tensor("idxs", [QP, 2 + ncc], i32, kind="ExternalInput")
    # cols 0..2121: host-built span (zeros + 10*flip slots); then clip vals
    tab = nc.dram_tensor("tab", [QP, width], f16, kind="ExternalInput")

    nfill = 32  # 43008 rows / 1344 rows per fill DMA
    with (
        nc.Block() as block,
        nc.semaphore("s_ix") as s_ix,
        nc.semaphore("s_kv") as s_kv,
        nc.semaphore("s_d") as s_d,
        nc.semaphore("s_z") as s_z,
        nc.semaphore("s_f") as s_f,
        nc.sbuf_tensor("idx_t", [QP, 2 + ncc], i32) as idx_t,
        nc.sbuf_tensor("tab_t", [QP, width], f16) as tab_t,
        nc.sbuf_tensor("zt", [128, 2772], f16) as zt,
    ):

        @block.sync
        def _(sync):
            sync.dma_start(out=idx_t[:], in_=idxs[:]).then_inc(s_ix, 16)
            if zero_fill:
                sync.wait_ge(s_z, 1)
                blk = 1344  # 1344*264*2B = 0.71 MB per fill DMA
                for i in range(nfill):
                    sync.dma_start(
                        out=out[i * blk:(i + 1) * blk, :], in_=zt[:, :]
                    ).then_inc(s_f, 16)
                drows = orows - ROWS
                half = drows // 2
                sync.dma_start(
                    out=out[ROWS:ROWS + half, :], in_=zt[:half, :WPAD]
                ).then_inc(s_f, 16)
                sync.dma_start(
                    out=out[ROWS + half:orows, :],
                    in_=zt[:drows - half, :WPAD],
                ).then_inc(s_f, 16)

        @block.scalar
        def _(scalar):
            scalar.dma_start(out=tab_t[:], in_=tab[:]).then_inc(s_kv, 16)

        if zero_fill:

            @block.vector
            def _(vector):
                vector.memset(zt[:], 0.0).then_inc(s_z, 1)

        @block.gpsimd
        def _(g):
            g.wait_ge(s_ix, 16)
            g.wait_ge(s_kv, 16)
            if zero_fill:
                g.wait_ge(s_f, (nfill + 2) * 16)
            g.indirect_dma_start(
                out=out[:],
                out_offset=bass.IndirectOffsetOnAxis(ap=idx_t[:, 0:1], axis=1),
                in_=tab_t[:, :SPAN],
                in_offset=None,
            ).then_inc(s_d, 16)
            g.indirect_dma_start(
                out=out[:],
                out_offset=bass.IndirectOffsetOnAxis(ap=idx_t[:42, 1:2],
                                                     axis=1),
                in_=tab_t[:42, :SPAN],
                in_offset=None,
            ).then_inc(s_d, 16)
            for j in range(ncc):
                base = SPAN + KS * j
                g.indirect_dma_start(
                    out=out[:],
                    out_offset=bass.IndirectOffsetOnAxis(
                        ap=idx_t[:, 2 + j:3 + j], axis=1
                    ),
                    in_=tab_t[:, base:base + KS],
                    in_offset=None,
                ).then_inc(s_d, 16)
            g.wait_ge(s_d, (2 + ncc) * 16)

    return nc


def _get_nc(zero_fill: bool, n_clip_cols: int):
    key = (bool(zero_fill), n_clip_cols)
    if key not in _NC_CACHE:
        nc = _build_nc(zero_fill, n_clip_cols)
        if not nc.is_finalized():
            nc.finalize()
        _NC_CACHE[key] = nc
    return _NC_CACHE[key]


def _prep_core(xc, flip10, n_clip_cols):
    """Host-fused indices + clip-value table for one core.

    Returns (idxs[126, 2+ncc] i32, clipvals[126, 9*ncc] f32) or None if the
    clip rows overflow n_clip_cols*126 slots (caller retries with more)."""
    ncc = n_clip_cols
    nslots = ncc * QP
    idxs = np.empty((QP, 2 + ncc), np.int32)
    idxs[:, 0] = DUMP + (np.arange(QP) % NPDUMP) * SPAN
    idxs[:, 1] = DUMP + (np.arange(QP) % NPDUMP) * SPAN
    for j in range(ncc):
        idxs[:, 2 + j] = RDUMP + (j * QP + np.arange(QP)) * KS
    clipvals = np.zeros((QP, KS * ncc), np.float32)
    clip_i = []
    clip_v = []
    ndump = 0
    for p in range(NPTS):
        r, c = int(xc[p, 0]), int(xc[p, 1])
        start = WPAD * (H * p + r - PAD) + c
        if PAD <= r <= H - 1 - PAD:
            if p < QP:
                idxs[p, 0] = start
            else:
                idxs[p - QP, 1] = start
        else:
            # whole patch dumps; visible rows go through the clip calls
            if p < QP:
                idxs[p, 0] = DUMP + (ndump % NPDUMP) * SPAN
            else:
                idxs[p - QP, 1] = DUMP + (ndump % NPDUMP) * SPAN
            ndump += 1
            for t in range(KS):
                rp = r - PAD + t
                if 0 <= rp < H:
                    clip_i.append(WPAD * (H * p + rp) + c)
                    clip_v.append(flip10[t])
    if len(clip_i) > nslots or ndump > NPDUMP:
        return None
    for k, (ci, cv) in enumerate(zip(clip_i, clip_v)):
        j, q = divmod(k, QP)
        idxs[q, 2 + j] = ci
        clipvals[q, KS * j:KS * (j + 1)] = cv
    return idxs, clipvals


def _in_maps(x, kernel2d):
    x = np.asarray(x)
    flip10 = 10.0 * np.asarray(kernel2d, dtype=np.float32)[::-1, ::-1]
    xr = x.reshape(NCORES, NPTS, 2)
    ncc = 1
    while True:
        preps = [_prep_core(xr[ci], flip10, ncc) for ci in range(NCORES)]
        if all(p is not None for p in preps):
            break
        ncc += 1
        assert ncc <= 6, "clip-row capacity exceeded (impossible for H=256)"
    # span content: zeros with the 9 flip10 rows at k*WPAD (same every core)
    span = np.zeros(SPAN, np.float16)
    for k in range(KS):
        span[k * WPAD:k * WPAD + KS] = flip10[k].astype(np.float16)
    maps = []
    for idxs, clipvals in preps:
        tab = np.empty((QP, SPAN + KS * ncc), np.float16)
        tab[:, :SPAN] = span[None, :]
        tab[:, SPAN:] = clipvals.astype(np.float16)
        maps.append({"idxs": idxs, "tab": tab})
    return ncc, maps


def _assemble(results):
    full = np.empty((B, KP, H, H), np.float32)
    for ci, res in enumerate(results):
        o = res["out"][:ROWS].reshape(BLOC, KP, H, WPAD)
        full[ci * BLOC:(ci + 1) * BLOC] = o[:, :, :, PAD:PAD + H]
    return full


def _run(ncc, zero_fill, maps, **kw):
    from concourse.bass_utils import run_bass_kernel_spmd

    nc = _get_nc(zero_fill, ncc)
    return run_bass_kernel_spmd(nc, maps, core_ids=list(range(NCORES)), **kw)


def _zero_contract_ok(x, results):
    """Sample must-be-zero cells to confirm outputs arrived pre-zeroed."""
    x = np.asarray(x).reshape(NCORES, NPTS, 2)
    rng = np.random.RandomState(0)
    for c in (0, NCORES - 1):
        o = results[c]["out"][:ROWS].reshape(NPTS, H, WPAD)
        for p in rng.choice(NPTS, 24, replace=False):
            r = x[c, p, 0]
            rows = np.arange(H)
            far = rows[(rows < r - PAD - 1) | (rows > r + PAD + 1)]
            sel = rng.choice(far, 8, replace=False)
            if np.any(o[p][sel] != 0.0):
                return False
    return True


def kernel(x, kernel2d):
    ncc, maps = _in_maps(x, kernel2d)
    res = _run(ncc, False, maps)
    if not _zero_contract_ok(x, res.results):
        # pre-zeroed-output contract failed; redo with explicit zero fill
        res = _run(ncc, True, maps)
    return _assemble(res.results)


# revision 6
# speedup vs baseline: 1.2693x; 1.1011x over previous
"""Trainium2 Bass kernel for nn_HeatmapBatch.

Reference computes: one-hot delta (value 10.0) per (batch, keypoint) at
integer coords (r, c) in a 256x256 image, then depthwise-convolves with a
shared 9x9 kernel.  Since each image holds exactly one delta, the output is
zeros everywhere except a 9x9 patch of 10*kernel2d[::-1,::-1] (XLA conv is
cross-correlation) centred at (r, c), clipped at the borders.

Device strategy (data-parallel over batch, 8 cores x 8 batches = 168
images per core):
  - Output per core is a column-padded [168*256 (+dump), 264] tensor in
    FP16 (rel tolerance is 2e-2; fp16 rounds at ~5e-4) so the scatter
    moves half the bytes; 4 pad columns each side so a patch row never
    wraps to the next row.  Host converts back to f32 and strips padding.
  - The runtime hands kernels pre-zeroed ExternalOutput buffers (documented
    contract in bass_utils/bass2jax), so the kernel only scatters patches.
  - A whole unclipped patch (rows r-4..r+4) is one contiguous 2121-element
    span of the padded image: 9 K-rows separated by 255 zeros.  Overwriting
    those gap zeros with zeros is harmless, so one indirect-DMA descriptor
    per patch suffices (the HW DGE pairs ONE offset per SBUF partition):
    2 scatter calls cover 168 patches (126+42 partitions).  Patches clipped
    at the top/bottom border are redirected to per-patch dump slots and
    their valid rows written by a third scatter call (9-element chunks,
    dump-padded slots; more calls only if >126 clipped rows, which needs
    >31 clipped points and never happens for uniform coords).
  - The ENTIRE span content (gap zeros + 10*flip value slots) is built on
    host and DMA'd straight into SBUF, so no engine ever touches the data
    between the input DMA and the scatter: no vector ops, no memset, no
    SBUF read-after-DMA races (the only consumers are the scatter packets,
    which start >1.3us after the descriptor-gen waits on the DMA sem).
  - Input DMAs issue in parallel (idx on the sync HW-DGE queue, span table
    on the scalar HW-DGE queue); desc-gen waits idx first, then the bigger
    table sem, so idx data has extra landing slack before the DGE reads it.
  - A fallback variant zero-fills the output with big DMAs first, in case
    the pre-zeroed-output contract ever fails (detected by sampling).
Host does sharding/layout prep and the final gather/strip of the padding.
"""

import numpy as np


def _ensure_axon_hooks():
    """bass_utils imports antenv.axon_hooks when tracing is requested (e.g.
    BASS_TRACE=1 in the environment); some images lack that module.  Provide
    it best-effort so a tracing harness degrades gracefully instead of
    crashing.  Never raises."""
    try:
        import antenv.axon_hooks  # noqa: F401
        return
    except Exception:
        pass
    try:
        import sys
        import types

        import antenv

        mod = types.ModuleType("antenv.axon_hooks")
        _state = {"hook": None}
        mod.set_axon_ntff_profile_hook = lambda h: _state.__setitem__("hook", h)
        mod.get_axon_ntff_profile_hook = lambda: _state["hook"]
        sys.modules["antenv.axon_hooks"] = mod
        antenv.axon_hooks = mod
        try:
            from trn_agent_boot.trn_boot import _ntff_profile_via_ctypes

            mod.set_axon_ntff_profile_hook(
                _ntff_profile_via_ctypes("/opt/axon/libaxon_pjrt.so")
            )
        except Exception:
            pass
    except Exception:
        pass


_ensure_axon_hooks()

B, KP, H = 64, 21, 256
KS, PAD = 9, 4
NCORES = 8
BLOC = B // NCORES          # 8 batches per core
NPTS = BLOC * KP            # 168 images per core
QP = 126                    # partitions used per scatter call
WPAD = H + 2 * PAD          # 264 padded columns
ROWS = NPTS * H             # 43008 image rows per core
SPAN = 8 * WPAD + KS        # 2121: contiguous span of one unclipped patch
NPDUMP = 32                 # patch-sized dump slots (distinct: parallel HBM)
DUMP = ROWS * WPAD          # first element of the dump zone
RDUMP = DUMP + NPDUMP * SPAN    # row-slot dump area

_NC_CACHE = {}


def _layout(n_clip_cols):
    dump_elems = NPDUMP * SPAN + n_clip_cols * QP * KS
    drows = (dump_elems + WPAD - 1) // WPAD
    return ROWS + drows


def _build_nc(zero_fill: bool, n_clip_cols: int):
    """Raw Bass: parallel input DMAs, span scatter (126+42) + n_clip_cols
    clip-row scatter calls, manual semaphores.  No compute engines."""
    from concourse import bass, mybir

    nc = bass.Bass(target_bir_lowering=False)
    i32, f16 = mybir.dt.int32, mybir.dt.float16
    orows = _layout(n_clip_cols)
    ncc = n_clip_cols
    width = SPAN + KS * ncc
    out = nc.dram_tensor("out", [orows, WPAD], f16, kind="ExternalOutput")
    idxs = nc.dram_tensor("idxs", [QP, 2 + ncc], i32, kind="ExternalInput")
    # cols 0..2121: host-built span (zeros + 10*flip slots); then clip vals
    tab = nc.dram_tensor("tab", [QP, width], f16, kind="ExternalInput")

    nfill = 32  # 43008 rows / 1344 rows per fill DMA
    with (
        nc.Block() as block,
        nc.semaphore("s_ix") as s_ix,
        nc.semaphore("s_kv") as s_kv,
        nc.semaphore("s_d") as s_d,
        nc.semaphore("s_z") as s_z,
        nc.semaphore("s_f") as s_f,
        nc.sbuf_tensor("idx_t", [QP, 2 + ncc], i32) as idx_t,
        nc.sbuf_tensor("tab_t", [QP, width], f16) as tab_t,
        nc.sbuf_tensor("zt", [128, 2772], f16) as zt,
    ):

        @block.sync
        def _(sync):
            sync.dma_start(out=idx_t[:], in_=idxs[:]).then_inc(s_ix, 16)
            if zero_fill:
                sync.wait_ge(s_z, 1)
                blk = 1344  # 1344*264*2B = 0.71 MB per fill DMA
                for i in range(nfill):
                    sync.dma_start(
                        out=out[i * blk:(i + 1) * blk, :], in_=zt[:, :]
                    ).then_inc(s_f, 16)
                drows = orows - ROWS
                half = drows // 2
                sync.dma_start(
                    out=out[ROWS:ROWS + half, :], in_=zt[:half, :WPAD]
                ).then_inc(s_f, 16)
                sync.dma_start(
                    out=out[ROWS + half:orows, :],
                    in_=zt[:drows - half, :WPAD],
                ).then_inc(s_f, 16)

        if zero_fill:

            @block.vector
            def _(vector):
                vector.memset(zt[:], 0.0).then_inc(s_z, 1)

        @block.gpsimd
        def _(g):
            # tab load rides the SAME SWDGE queue as the scatters below: the
            # queue dispatches FIFO, so every tab packet is dispatched before
            # any scatter packet and the only in-flight overlap is between
            # the tail tab partitions (~110+) and the head scatter packets
            # (partitions 0..15) — disjoint SBUF rows.  No semaphore needed.
            g.dma_start(out=tab_t[:], in_=tab[:]).then_inc(s_kv, 16)
            g.wait_ge(s_ix, 16)
            if zero_fill:
                g.wait_ge(s_kv, 16)
                g.wait_ge(s_f, (nfill + 2) * 16)
            g.indirect_dma_start(
                out=out[:],
                out_offset=bass.IndirectOffsetOnAxis(ap=idx_t[:, 0:1], axis=1),
                in_=tab_t[:, :SPAN],
                in_offset=None,
            ).then_inc(s_d, 16)
            g.indirect_dma_start(
                out=out[:],
                out_offset=bass.IndirectOffsetOnAxis(ap=idx_t[:42, 1:2],
                                                     axis=1),
                in_=tab_t[:42, :SPAN],
                in_offset=None,
            ).then_inc(s_d, 16)
            for j in range(ncc):
                base = SPAN + KS * j
                g.indirect_dma_start(
                    out=out[:],
                    out_offset=bass.IndirectOffsetOnAxis(
                        ap=idx_t[:, 2 + j:3 + j], axis=1
                    ),
                    in_=tab_t[:, base:base + KS],
                    in_offset=None,
                ).then_inc(s_d, 16)
            # no final wait: the Block-exit dge_drain blocks until the SWDGE
            # queue (tab load + all scatters) has fully completed

    return nc


def _get_nc(zero_fill: bool, n_clip_cols: int):
    key = (bool(zero_fill), n_clip_cols)
    if key not in _NC_CACHE:
        nc = _build_nc(zero_fill, n_clip_cols)
        if not nc.is_finalized():
            nc.finalize()
        _NC_CACHE[key] = nc
    return _NC_CACHE[key]


def _prep_core(xc, flip10, n_clip_cols):
    """Host-fused indices + clip-value table for one core.

    Returns (idxs[126, 2+ncc] i32, clipvals[126, 9*ncc] f32) or None if the
    clip rows overflow n_clip_cols*126 slots (caller retries with more)."""
    ncc = n_clip_cols
    nslots = ncc * QP
    idxs = np.empty((QP, 2 + ncc), np.int32)
    idxs[:, 0] = DUMP + (np.arange(QP) % NPDUMP) * SPAN
    idxs[:, 1] = DUMP + (np.arange(QP) % NPDUMP) * SPAN
    for j in range(ncc):
        idxs[:, 2 + j] = RDUMP + (j * QP + np.arange(QP)) * KS
    clipvals = np.zeros((QP, KS * ncc), np.float32)
    clip_i = []
    clip_v = []
    ndump = 0
    for p in range(NPTS):
        r, c = int(xc[p, 0]), int(xc[p, 1])
        start = WPAD * (H * p + r - PAD) + c
        if PAD <= r <= H - 1 - PAD:
            if p < QP:
                idxs[p, 0] = start
            else:
                idxs[p - QP, 1] = start
        else:
            # whole patch dumps; visible rows go through the clip calls
            if p < QP:
                idxs[p, 0] = DUMP + (ndump % NPDUMP) * SPAN
            else:
                idxs[p - QP, 1] = DUMP + (ndump % NPDUMP) * SPAN
            ndump += 1
            for t in range(KS):
                rp = r - PAD + t
                if 0 <= rp < H:
                    clip_i.append(WPAD * (H * p + rp) + c)
                    clip_v.append(flip10[t])
    if len(clip_i) > nslots or ndump > NPDUMP:
        return None
    for k, (ci, cv) in enumerate(zip(clip_i, clip_v)):
        j, q = divmod(k, QP)
        idxs[q, 2 + j] = ci
        clipvals[q, KS * j:KS * (j + 1)] = cv
    return idxs, clipvals


def _in_maps(x, kernel2d):
    x = np.asarray(x)
    flip10 = 10.0 * np.asarray(kernel2d, dtype=np.float32)[::-1, ::-1]
    xr = x.reshape(NCORES, NPTS, 2)
    ncc = 1
    while True:
        preps = [_prep_core(xr[ci], flip10, ncc) for ci in range(NCORES)]
        if all(p is not None for p in preps):
            break
        ncc += 1
        assert ncc <= 6, "clip-row capacity exceeded (impossible for H=256)"
    # span content: zeros with the 9 flip10 rows at k*WPAD (same every core)
    span = np.zeros(SPAN, np.float16)
    for k in range(KS):
        span[k * WPAD:k * WPAD + KS] = flip10[k].astype(np.float16)
    maps = []
    for idxs, clipvals in preps:
        tab = np.empty((QP, SPAN + KS * ncc), np.float16)
        tab[:, :SPAN] = span[None, :]
        tab[:, SPAN:] = clipvals.astype(np.float16)
        maps.append({"idxs": idxs, "tab": tab})
    return ncc, maps


def _assemble(results):
    full = np.empty((B, KP, H, H), np.float32)
    for ci, res in enumerate(results):
        o = res["out"][:ROWS].reshape(BLOC, KP, H, WPAD)
        full[ci * BLOC:(ci + 1) * BLOC] = o[:, :, :, PAD:PAD + H]
    return full


def _run(ncc, zero_fill, maps, **kw):
    from concourse.bass_utils import run_bass_kernel_spmd

    nc = _get_nc(zero_fill, ncc)
    return run_bass_kernel_spmd(nc, maps, core_ids=list(range(NCORES)), **kw)


def _zero_contract_ok(x, results):
    """Sample must-be-zero cells to confirm outputs arrived pre-zeroed."""
    x = np.asarray(x).reshape(NCORES, NPTS, 2)
    rng = np.random.RandomState(0)
    for c in (0, NCORES - 1):
        o = results[c]["out"][:ROWS].reshape(NPTS, H, WPAD)
        for p in rng.choice(NPTS, 24, replace=False):
            r = x[c, p, 0]
            rows = np.arange(H)
            far = rows[(rows < r - PAD - 1) | (rows > r + PAD + 1)]
            sel = rng.choice(far, 8, replace=False)
            if np.any(o[p][sel] != 0.0):
                return False
    return True


def kernel(x, kernel2d):
    ncc, maps = _in_maps(x, kernel2d)
    res = _run(ncc, False, maps)
    if not _zero_contract_ok(x, res.results):
        # pre-zeroed-output contract failed; redo with explicit zero fill
        res = _run(ncc, True, maps)
    return _assemble(res.results)
